# revision 1
# baseline (speedup 1.0000x reference)
"""Trainium2 Bass kernel for a causal self-attention block.

Reference computation (fp32):
    qkv = x @ W_qkv + b_qkv ; q,k,v = split(qkv)
    scores = (q @ k.T + mask) / sqrt(hd)
    wts = exp(scores) / (sum(exp(scores)) + 1e-9)
    y = (wts @ v) @ W_out + b_out
    out = LayerNorm(x + y) * gamma + beta

Sharding: 8 cores = 4 batches x 2 query-shards.  Each core computes the
full attention block for one batch and half of the query positions.  The
query chunks are interleaved between the two shards of a batch so the
causal block-skipping work is balanced, and the block pattern (which
k-tiles are needed / need a mask add) is derived from the actual mask on
the host, so non-causal masks degrade gracefully to the dense kernel.

On-device layout is feature-major (x^T / K^T / Q^T / attn^T / y^T), with
V kept token-major with a per-head all-ones column so the softmax
denominator falls out of the attention matmul for free.  LayerNorm
reductions over the feature (partition) axis are done with ones-vector
matmuls.  Device transposes are avoided entirely: the host feeds pre-
transposed inputs and transposes the per-core outputs back.

Matmul operands use float32r (single-pass fp32 PE mode, 4x the fp32
rate); every tensor consumed by an fp32r matmul is declared float32r
along its whole producer chain to satisfy the BIR verifier.  Q/K/V/z are
staged through per-core HBM scratch so the static SBUF pool footprint
stays under the 192KB/partition budget; pools are tag-shared across
phases.
"""

import numpy as np
import ml_dtypes

import concourse.bass as bass
import concourse.tile as tile
from concourse import bacc, mybir
from concourse.bass_utils import run_bass_kernel_spmd

# Problem dims (hardcoded per harness contract)
B, S, D, H = 4, 2048, 1024, 16
N_CORES = 8
QCHUNK = 256                     # q columns per position slot
KTILE = 128                      # k rows per tile
NEG_THRESH = -1.0e8              # mask <= this  =>  exp((qk+m)/8) == 0 in fp32
SM_EPS = 1.0e-9
LN_EPS = 1.0e-5

F32 = mybir.dt.float32
F32R = mybir.dt.float32r
BF16 = mybir.dt.bfloat16

ALU = mybir.AluOpType
ACTF = mybir.ActivationFunctionType


# ----------------------------------------------------------------------------
# Host-side mask analysis / shard assignment
# ----------------------------------------------------------------------------

def _mask_pattern(attn_mask, b_, s_):
    """Classify mask blocks and build a core-uniform block schedule.

    Returns (chunk_at[b, shard, pos], kt_lists[pos] -> list of ktiles,
    tt_blocks -> ordered list of (pos, ktile) needing a mask add).
    """
    nq = s_ // QCHUNK
    nk = s_ // KTILE
    m = attn_mask.reshape(b_, nq, QCHUNK, nk, KTILE)
    bmax = m.max(axis=(2, 4))
    bmin = m.min(axis=(2, 4))
    skip = bmax <= NEG_THRESH                    # contributes exactly 0
    zero = (bmin == 0.0) & (bmax == 0.0)         # no mask add needed
    needed = ~skip                               # [b, nq, nk]
    counts = needed.sum(axis=2)                  # [b, nq]

    order = np.argsort(-counts, axis=1, kind="stable")   # chunks by desc count
    npos = nq // 2
    chunk_at = np.zeros((b_, 2, npos), np.int64)
    chunk_at[:, 0, :] = order[:, 0::2]
    chunk_at[:, 1, :] = order[:, 1::2]

    kt_lists = []
    tt_blocks = []
    un_list = []
    tt_list = []
    for p in range(npos):
        un = np.zeros(nk, bool)
        for bb in range(b_):
            for sh in range(2):
                un |= needed[bb, chunk_at[bb, sh, p]]
        any_tt = np.zeros(nk, bool)
        for bb in range(b_):
            for sh in range(2):
                c = chunk_at[bb, sh, p]
                # mask add needed wherever a core computes the block but its
                # mask is not identically zero there (incl. dead padding)
                any_tt |= un & ~(needed[bb, c] & zero[bb, c])
        kt_lists.append(np.nonzero(un)[0].tolist())
        un_list.append(un)
        tt_list.append(any_tt)
    # PSUM accumulation groups are per 512-col bank: fuse position pairs into
    # one group by unioning their k-tile lists; the extra blocks are masked.
    for j in range(0, npos - 1, 2):
        un2 = un_list[j] | un_list[j + 1]
        kt_lists[j] = kt_lists[j + 1] = np.nonzero(un2)[0].tolist()
        for p in (j, j + 1):
            # recompute per-core tt for the widened list
            any_tt = np.zeros(nk, bool)
            for bb in range(b_):
                for sh in range(2):
                    c = chunk_at[bb, sh, p]
                    any_tt |= un2 & ~(needed[bb, c] & zero[bb, c])
            tt_list[p] = any_tt
    for p in range(npos):
        for t in kt_lists[p]:
            if tt_list[p][t]:
                tt_blocks.append((p, t))
    return chunk_at, kt_lists, tt_blocks


# ----------------------------------------------------------------------------
# Device program
# ----------------------------------------------------------------------------

def _build_program(dims, kt_lists, tt_blocks, mm_dt=F32R, att_dt=BF16,
                   n_iters=1, timing_mode=False,
                   phases=("p1", "p2", "p34")):
    """Emit the per-core Bass/Tile program (identical for all cores)."""
    b_, s_, d_, h_ = dims
    hd = d_ // h_
    he = hd + 1                   # head dim + ones column
    sq = s_ // 2                  # queries per core
    npos = sq // QCHUNK
    nf = d_ // 128                # feature tiles
    nst = s_ // 128               # s tiles (token-major V) == n ktiles
    ndc = d_ // 128               # contraction d tiles
    qw = min(512, sq)             # q matmul chunk width
    sw = min(512, s_)             # s streaming width for K/V build
    fw = min(512, d_)             # f chunk width for V build
    kw = min(1024, s_)            # kh streaming piece width
    ntt = max(1, len(tt_blocks))
    tt_idx = {pt: i for i, pt in enumerate(tt_blocks)}
    kt_sets = [set(k) for k in kt_lists]
    all_kt = sorted(set().union(*kt_sets)) if kt_lists else []

    nc = bacc.Bacc("TRN2", target_bir_lowering=False, debug=False,
                   num_devices=N_CORES)

    # I/O.  Tensors feeding fp32r matmuls are declared mm_dt end-to-end.
    # In timing_mode the big inputs are internal DRAM scratch (uninitialised)
    # so per-call host->device transfer stays tiny; timing is data-independent.
    big = "Internal" if timing_mode else "ExternalInput"
    xT = nc.dram_tensor("xT", [d_, s_], mm_dt, kind=big).ap()
    xqT = nc.dram_tensor("xqT", [d_, sq], mm_dt, kind=big).ap()
    wqkv = nc.dram_tensor("wqkv", [d_, 3 * d_], mm_dt, kind=big).ap()
    wout = nc.dram_tensor("wout", [d_, d_], mm_dt, kind=big).ap()
    bqkv = nc.dram_tensor("bqkv", [3 * d_, 1], F32, kind="ExternalInput").ap()
    bout = nc.dram_tensor("bout", [d_, 1], F32, kind="ExternalInput").ap()
    gamma = nc.dram_tensor("gamma", [d_, 1], F32, kind="ExternalInput").ap()
    beta = nc.dram_tensor("beta", [d_, 1], F32, kind="ExternalInput").ap()
    bvrep = nc.dram_tensor("bvrep", [128, d_], F32, kind="ExternalInput").ap()
    maskT = nc.dram_tensor("maskT", [ntt, KTILE, QCHUNK], att_dt,
                           kind="ExternalInput").ap()
    ones2d = nc.dram_tensor("ones2d", [128, 128], mm_dt,
                            kind="ExternalInput").ap()
    onesb = nc.dram_tensor("onesb", [128, 128], att_dt,
                           kind="ExternalInput").ap()
    yT = nc.dram_tensor("yT", [d_, sq], F32, kind="ExternalOutput").ap()

    # spills (per-core private HBM)
    qspill = nc.dram_tensor("qspill", [d_, sq], att_dt).ap()
    kspill = nc.dram_tensor("kspill", [d_, s_], att_dt).ap()
    aspill = nc.dram_tensor("aspill", [d_, sq], mm_dt).ap()
    zspill = nc.dram_tensor("zspill", [d_, sq], mm_dt).ap()

    def emit(tc, P):
        ctx_lp = nc.allow_low_precision(reason="fp32r matmul operand chain")
        ctx_lp.__enter__()
        U = P["u"]

        # ---- constants ------------------------------------------------
        bq_sb = [U.tile([128, 1], F32, name="bq", tag="bq", bufs=nf)
                 for _ in range(nf)]
        bk_sb = [U.tile([128, 1], F32, name="bk", tag="bk", bufs=nf)
                 for _ in range(nf)]
        bo_sb = [U.tile([128, 1], F32, name="bo", tag="bo", bufs=nf)
                 for _ in range(nf)]
        ga_sb = [U.tile([128, 1], F32, name="ga", tag="ga", bufs=nf)
                 for _ in range(nf)]
        be_sb = [U.tile([128, 1], F32, name="be", tag="be", bufs=nf)
                 for _ in range(nf)]
        for f in range(nf):
            nc.sync.dma_start(bq_sb[f][:], bqkv[f * 128:(f + 1) * 128, :])
            nc.sync.dma_start(bk_sb[f][:], bqkv[d_ + f * 128:d_ + (f + 1) * 128, :])
            nc.sync.dma_start(bo_sb[f][:], bout[f * 128:(f + 1) * 128, :])
            nc.sync.dma_start(ga_sb[f][:], gamma[f * 128:(f + 1) * 128, :])
            nc.sync.dma_start(be_sb[f][:], beta[f * 128:(f + 1) * 128, :])
        bv_sb = U.tile([128, d_], F32, name="bv", tag="bv", bufs=1)
        nc.sync.dma_start(bv_sb[:], bvrep[:])
        ones_sb = U.tile([128, 1], mm_dt, name="ones", tag="ones", bufs=1)
        nc.sync.dma_start(ones_sb[:], ones2d[:, 0:1])
        onesr_sb = U.tile([1, 128], mm_dt, name="onesr", tag="onesr", bufs=1)
        nc.sync.dma_start(onesr_sb[:], ones2d[0:1, :])
        eps_sb = U.tile([1, 1], F32, name="eps", tag="eps", bufs=1)
        nc.vector.memset(eps_sb[:], LN_EPS)

        def bcast(dst_sb, row_sb, nrows, ncols):
            """Replicate a [1, ncols] SBUF row across nrows partitions."""
            rp = P["psA"].tile([128, 1024], F32, name="psA", tag="psA")
            for j in range(0, ncols, 512):
                jw = min(512, ncols - j)
                nc.tensor.matmul(rp[0:nrows, j:j + jw],
                                 onesr_sb[:, 0:nrows],
                                 row_sb[:, j:j + jw], start=True, stop=True)
            nc.scalar.activation(dst_sb, rp[0:nrows, 0:ncols], ACTF.Copy)

        mask_sb = [U.tile([KTILE, QCHUNK], att_dt, name="mk", tag="mk", bufs=ntt)
                   for _ in range(ntt)]
        for i in range(ntt):
            nc.sync.dma_start(mask_sb[i][:], maskT[i])

        do_p1 = "p1" in phases
        do_p2 = "p2" in phases
        do_p34 = "p34" in phases
        # ---- P1a: Q^T projection (feature-major), spill to HBM --------
        if do_p1:
            wq = [U.tile([128, d_], mm_dt, name="opA", tag="opA", bufs=ndc)
                  for _ in range(ndc)]
            for dc in range(ndc):
                nc.sync.dma_start(wq[dc][:], wqkv[dc * 128:(dc + 1) * 128, 0:d_])
            xq_sb = [U.tile([128, sq], mm_dt, name="opB", tag="opB", bufs=ndc + 1)
                     for _ in range(ndc)]
            for dc in range(ndc):
                nc.sync.dma_start(xq_sb[dc][:], xqT[dc * 128:(dc + 1) * 128, :])
            for f in range(nf):
                for c in range(sq // qw):
                    ps = P["psA"].tile([128, 1024], F32, name="psA", tag="psA")
                    for dc in range(ndc):
                        nc.tensor.matmul(
                            ps[:, 0:qw],
                            wq[dc][:, f * 128:(f + 1) * 128],
                            xq_sb[dc][:, c * qw:(c + 1) * qw],
                            start=(dc == 0), stop=(dc == ndc - 1))
                    st = U.tile([128, 512], att_dt, name="st", tag="st", bufs=2)
                    nc.vector.tensor_scalar_add(st[:, 0:qw], ps[:, 0:qw],
                                                bq_sb[f][:])
                    nc.sync.dma_start(
                        qspill[f * 128:(f + 1) * 128, c * qw:(c + 1) * qw],
                        st[:, 0:qw])

        # ---- P1b-K: K^T projection, spill ------------------------------
        if do_p1:
            wk = [U.tile([128, d_], mm_dt, name="opA", tag="opA", bufs=ndc)
                  for _ in range(ndc)]
            for dc in range(ndc):
                nc.sync.dma_start(wk[dc][:], wqkv[dc * 128:(dc + 1) * 128, d_:2 * d_])
            for quarter in range(s_ // sw):
                xt_sb = [U.tile([128, sw], mm_dt, name="opB", tag="opB",
                                bufs=ndc + 1) for _ in range(ndc)]
                for dc in range(ndc):
                    nc.sync.dma_start(
                        xt_sb[dc][:],
                        xT[dc * 128:(dc + 1) * 128, quarter * sw:(quarter + 1) * sw])
                for f in range(nf):
                    ps = P["psA"].tile([128, 1024], F32, name="psA", tag="psA")
                    for dc in range(ndc):
                        nc.tensor.matmul(
                            ps[:, 0:sw],
                            wk[dc][:, f * 128:(f + 1) * 128],
                            xt_sb[dc][:],
                            start=(dc == 0), stop=(dc == ndc - 1))
                    st = U.tile([128, 512], att_dt, name="st", tag="st", bufs=2)
                    nc.vector.tensor_scalar_add(st[:, 0:sw], ps[:, 0:sw],
                                                bk_sb[f][:])
                    nc.sync.dma_start(
                        kspill[f * 128:(f + 1) * 128,
                               quarter * sw:(quarter + 1) * sw], st[:, 0:sw])

        # ---- P1b-V: token-major V with ones column, spill ---------------
        v_sb = [U.tile([128, h_, he], att_dt, name="vv", tag="vv", bufs=nst)
                for _ in range(nst)]
        if do_p1:
            wv = [U.tile([128, d_], mm_dt, name="opA", tag="opA", bufs=ndc)
                  for _ in range(ndc)]
            for dc in range(ndc):
                nc.sync.dma_start(wv[dc][:],
                                  wqkv[dc * 128:(dc + 1) * 128, 2 * d_:3 * d_])
            for quarter in range(s_ // sw):
                xt_sb = [U.tile([128, sw], mm_dt, name="opB", tag="opB",
                                bufs=ndc + 1) for _ in range(ndc)]
                for dc in range(ndc):
                    nc.sync.dma_start(
                        xt_sb[dc][:],
                        xT[dc * 128:(dc + 1) * 128, quarter * sw:(quarter + 1) * sw])
                for sl in range(sw // 128):
                    st_i = quarter * (sw // 128) + sl
                    vt = v_sb[st_i]
                    for fc in range(d_ // fw):
                        ps = P["psA"].tile([128, 1024], F32, name="psA", tag="psA")
                        for dc in range(ndc):
                            nc.tensor.matmul(
                                ps[:, 0:fw],
                                xt_sb[dc][:, sl * 128:(sl + 1) * 128],
                                wv[dc][:, fc * fw:(fc + 1) * fw],
                                start=(dc == 0), stop=(dc == ndc - 1))
                        nc.vector.tensor_add(
                            vt[:, fc * (fw // hd):(fc + 1) * (fw // hd), 0:hd],
                            ps[:, 0:fw].rearrange("p (h e) -> p h e", e=hd),
                            bv_sb[:, fc * fw:(fc + 1) * fw]
                                .rearrange("p (h e) -> p h e", e=hd))
                    nc.sync.dma_start(
                        vt[:, :, hd:hd + 1],
                        onesb[:, 0:h_].rearrange("p (h o) -> p h o", o=1))

        # ---- P2: attention --------------------------------------------
        if do_p2:
            inv_sqrt_hd = 1.0 / float(np.sqrt(hd))
            npieces = s_ // kw
            for h in range(h_):
                kh = [U.tile([hd, kw], att_dt, name="kh", tag="kh", bufs=2)
                      for _ in range(npieces)]
                for pc in range(npieces):
                    nc.sync.dma_start(
                        kh[pc][:],
                        kspill[h * hd:(h + 1) * hd, pc * kw:(pc + 1) * kw])
                qh = U.tile([hd, sq], att_dt, name="qh", tag="qh", bufs=2)
                nc.sync.dma_start(qh[:], qspill[h * hd:(h + 1) * hd, :])
                att_ps = P["psB"].tile([hd + 1, npos * QCHUNK], F32,
                                       name="psB", tag="psB")
                first = {j: True for j in range(0, npos, 2)}
                pairs = list(range(0, npos, 2))
                for t in all_kt:
                    plist = [p for p in range(npos) if t in kt_sets[p]]
                    if not plist:
                        continue
                    pc, to = t // (kw // KTILE), t % (kw // KTILE)
                    sc_ps = P["psA"].tile([128, 1024], F32, name="psA", tag="psA")
                    for p in plist:
                        nc.tensor.matmul(
                            sc_ps[:, p * QCHUNK:(p + 1) * QCHUNK],
                            kh[pc][:, to * KTILE:(to + 1) * KTILE],
                            qh[:, p * QCHUNK:(p + 1) * QCHUNK],
                            start=True, stop=True)
                    num = U.tile([128, npos * QCHUNK], att_dt, name="num",
                                 tag="num", bufs=2)
                    runs = []            # exp over contiguous runs of positions
                    for p in plist:
                        if runs and runs[-1][1] == p:
                            runs[-1][1] = p + 1
                        else:
                            runs.append([p, p + 1])
                    for r0, r1 in runs:
                        nc.scalar.activation(
                            num[:, r0 * QCHUNK:r1 * QCHUNK],
                            sc_ps[:, r0 * QCHUNK:r1 * QCHUNK],
                            ACTF.Exp, scale=inv_sqrt_hd)
                    # multiplicative mask exp(m/sqrt(hd)) on SBUF (never RMW PSUM:
                    # concurrent PE writes to the same PSUM bank are fatal)
                    for p in plist:
                        if (p, t) in tt_idx:
                            mi = tt_idx[(p, t)]
                            nc.vector.tensor_mul(
                                num[:, p * QCHUNK:(p + 1) * QCHUNK],
                                num[:, p * QCHUNK:(p + 1) * QCHUNK],
                                mask_sb[mi][:])
                    for j in pairs:
                        if t not in kt_sets[j]:
                            continue
                        pw = min(2, npos - j) * QCHUNK
                        nc.tensor.matmul(
                            att_ps[:, j * QCHUNK:j * QCHUNK + pw],
                            v_sb[t][:, h, :],
                            num[:, j * QCHUNK:j * QCHUNK + pw],
                            start=first[j], stop=(t == kt_lists[j][-1]))
                        first[j] = False
                # epilogue: normalize by (denom + eps)
                den = U.tile([1, npos * QCHUNK], mm_dt, name="den", tag="den",
                             bufs=2)
                nc.vector.tensor_scalar_add(den[:], att_ps[hd:hd + 1, :], SM_EPS)
                nc.vector.reciprocal(den[:], den[:])
                rep = U.tile([hd, npos * QCHUNK], F32, name="rp", tag="rp", bufs=2)
                bcast(rep[:], den[:], hd, npos * QCHUNK)
                anorm = U.tile([hd, npos * QCHUNK], mm_dt, name="an", tag="an",
                               bufs=2)
                nc.vector.tensor_mul(anorm[:], att_ps[0:hd, :], rep[:])
                nc.sync.dma_start(aspill[h * hd:(h + 1) * hd, :], anorm[:])

        # ---- P3: out-projection + bias + residual + LN stats ----------
        if do_p34:
            wo = [U.tile([128, d_], mm_dt, name="opA", tag="opA", bufs=ndc)
                  for _ in range(ndc)]
            for dc in range(ndc):
                nc.sync.dma_start(wo[dc][:], wout[dc * 128:(dc + 1) * 128, :])
            at_sb = [U.tile([128, sq], mm_dt, name="opB", tag="opB", bufs=ndc + 1)
                     for _ in range(ndc)]
            for dc in range(ndc):
                nc.sync.dma_start(at_sb[dc][:], aspill[dc * 128:(dc + 1) * 128, :])
            sum_ps = P["psB"].tile([1, sq], F32, name="psB", tag="psB")
            ssq_ps = P["psB"].tile([1, sq], F32, name="psB", tag="psB")
            for f in range(nf):
                zt = U.tile([128, sq], mm_dt, name="zz", tag="zz", bufs=3)
                xq2 = U.tile([128, sq], F32, name="ln", tag="ln", bufs=3)
                nc.sync.dma_start(xq2[:],
                                  xqT[f * 128:(f + 1) * 128, :].bitcast(F32))
                for c in range(sq // qw):
                    ps = P["psA"].tile([128, 1024], F32, name="psA", tag="psA")
                    for dc in range(ndc):
                        nc.tensor.matmul(
                            ps[:, 0:qw],
                            wo[dc][:, f * 128:(f + 1) * 128],
                            at_sb[dc][:, c * qw:(c + 1) * qw],
                            start=(dc == 0), stop=(dc == ndc - 1))
                    nc.vector.scalar_tensor_tensor(
                        zt[:, c * qw:(c + 1) * qw],
                        in0=ps[:, 0:qw],
                        scalar=bo_sb[f][:],
                        in1=xq2[:, c * qw:(c + 1) * qw],
                        op0=ALU.add, op1=ALU.add)
                sqr = U.tile([128, sq], mm_dt, name="ln", tag="ln", bufs=3)
                nc.scalar.activation(sqr[:], zt[:], ACTF.Square)
                for c in range(sq // qw):
                    nc.tensor.matmul(sum_ps[0:1, c * qw:(c + 1) * qw],
                                     ones_sb[:],
                                     zt[:, c * qw:(c + 1) * qw],
                                     start=(f == 0), stop=(f == nf - 1))
                    nc.tensor.matmul(ssq_ps[0:1, c * qw:(c + 1) * qw],
                                     ones_sb[:],
                                     sqr[:, c * qw:(c + 1) * qw],
                                     start=(f == 0), stop=(f == nf - 1))
                nc.sync.dma_start(zspill[f * 128:(f + 1) * 128, :], zt[:])

        # ---- P4: LayerNorm normalize -----------------------------------
        if do_p34:
            lmean = U.tile([1, sq], mm_dt, name="lmean", tag="lmean", bufs=1)
            lrstd = U.tile([1, sq], mm_dt, name="lrstd", tag="lrstd", bufs=1)
            msq = U.tile([1, sq], F32, name="msq", tag="lnsc", bufs=3)
            m2 = U.tile([1, sq], F32, name="m2", tag="lnsc", bufs=3)
            var = U.tile([1, sq], F32, name="var", tag="lnsc", bufs=3)
            sd = U.tile([1, sq], F32, name="sd", tag="lnsc", bufs=3)
            nc.vector.tensor_scalar_mul(lmean[:], sum_ps[0:1, :], 1.0 / d_)
            nc.vector.tensor_scalar_mul(msq[:], ssq_ps[0:1, :], 1.0 / d_)
            nc.vector.tensor_mul(m2[:], lmean[:].bitcast(F32),
                                 lmean[:].bitcast(F32))
            nc.vector.tensor_sub(var[:], msq[:], m2[:])
            nc.scalar.activation(sd[:], var[:], ACTF.Sqrt, bias=eps_sb[:])
            nc.vector.reciprocal(lrstd[:], sd[:])
            mrep = U.tile([128, sq], F32, name="rp", tag="rp", bufs=2)
            bcast(mrep[:], lmean[:], 128, sq)
            rrep = U.tile([128, sq], F32, name="rp2", tag="rp2", bufs=1)
            bcast(rrep[:], lrstd[:], 128, sq)
            for f in range(nf):
                zt = U.tile([128, sq], mm_dt, name="zz", tag="zz", bufs=3)
                nc.sync.dma_start(zt[:], zspill[f * 128:(f + 1) * 128, :])
                t1 = U.tile([128, sq], F32, name="ln", tag="ln", bufs=3)
                nc.vector.tensor_sub(t1[:], zt[:].bitcast(F32), mrep[:])
                nc.vector.tensor_mul(t1[:], t1[:], rrep[:])
                nc.vector.tensor_scalar(t1[:], t1[:], ga_sb[f][:], be_sb[f][:],
                                        ALU.mult, ALU.add)
                nc.sync.dma_start(yT[f * 128:(f + 1) * 128, :], t1[:])
        ctx_lp.__exit__(None, None, None)

    from contextlib import ExitStack
    with tile.TileContext(nc) as tc:
        with ExitStack() as ctx:
            P = {
                "u": ctx.enter_context(tc.tile_pool(name="u", bufs=2)),
                "psA": ctx.enter_context(
                    tc.tile_pool(name="psA", bufs=2, space="PSUM")),
                "psB": ctx.enter_context(
                    tc.tile_pool(name="psB", bufs=2, space="PSUM")),
            }
            if n_iters > 1:
                with tc.For_i(0, n_iters, 1):
                    emit(tc, P)
            else:
                emit(tc, P)
    nc.compile()
    return nc


# ----------------------------------------------------------------------------
# Host wrapper
# ----------------------------------------------------------------------------

_CACHE = {}


def _get_program(pattern_key, kt_lists, tt_blocks, n_iters=1,
                 timing_mode=False, phases=("p1", "p2", "p34")):
    key = (pattern_key, n_iters, timing_mode, tuple(phases))
    if key not in _CACHE:
        _CACHE[key] = _build_program((B, S, D, H), kt_lists, tt_blocks,
                                     n_iters=n_iters, timing_mode=timing_mode,
                                     phases=phases)
    return _CACHE[key]


def _prep_inputs(x, attn_mask, W_qkv, b_qkv, W_out, b_out, gamma, beta,
                 chunk_at, tt_blocks):
    b_, s_, d_ = x.shape
    f32 = np.float32
    in_maps = []
    qsels = []
    bvrep = np.ascontiguousarray(
        np.broadcast_to(b_qkv[2 * d_:3 * d_][None, :], (128, d_)), dtype=f32)
    wqkv_c = np.ascontiguousarray(W_qkv, dtype=f32)
    wout_c = np.ascontiguousarray(W_out, dtype=f32)
    bqkv_c = np.ascontiguousarray(b_qkv.reshape(3 * d_, 1), dtype=f32)
    bout_c = np.ascontiguousarray(b_out.reshape(d_, 1), dtype=f32)
    gamma_c = np.ascontiguousarray(gamma.reshape(d_, 1), dtype=f32)
    beta_c = np.ascontiguousarray(beta.reshape(d_, 1), dtype=f32)
    ones_c = np.ones((128, 128), f32)
    for core in range(N_CORES):
        bb, sh = core // 2, core % 2
        chunks = chunk_at[bb, sh]
        qsel = np.concatenate(
            [np.arange(c * QCHUNK, (c + 1) * QCHUNK) for c in chunks])
        qsels.append(qsel)
        xT_ = np.ascontiguousarray(x[bb].T, dtype=f32)
        xqT_ = np.ascontiguousarray(x[bb][qsel].T, dtype=f32)
        hd = d_ // H
        if tt_blocks:
            mt = np.stack([
                np.exp(np.ascontiguousarray(
                    attn_mask[bb,
                              chunks[p] * QCHUNK:(chunks[p] + 1) * QCHUNK,
                              t * KTILE:(t + 1) * KTILE].T,
                    dtype=np.float64) / np.sqrt(hd))
                for (p, t) in tt_blocks]).astype(ml_dtypes.bfloat16)
        else:
            mt = np.zeros((1, KTILE, QCHUNK), ml_dtypes.bfloat16)
        in_maps.append({
            "xT": xT_, "xqT": xqT_,
            "wqkv": wqkv_c, "wout": wout_c,
            "bqkv": bqkv_c, "bout": bout_c,
            "gamma": gamma_c, "beta": beta_c,
            "bvrep": bvrep, "maskT": mt,
            "ones2d": ones_c,
            "onesb": ones_c.astype(ml_dtypes.bfloat16),
        })
    return in_maps, qsels


def kernel(x, attn_mask, W_qkv, b_qkv, W_out, b_out, gamma, beta,
           n_iters=1):
    x = np.asarray(x, np.float32)
    attn_mask = np.asarray(attn_mask, np.float32)
    chunk_at, kt_lists, tt_blocks = _mask_pattern(attn_mask, B, S)
    pattern_key = (tuple(tuple(k) for k in kt_lists), tuple(tt_blocks))
    nc = _get_program(pattern_key, kt_lists, tt_blocks, n_iters=n_iters)
    in_maps, qsels = _prep_inputs(
        x, attn_mask, np.asarray(W_qkv), np.asarray(b_qkv),
        np.asarray(W_out), np.asarray(b_out), np.asarray(gamma),
        np.asarray(beta), chunk_at, tt_blocks)
    res = run_bass_kernel_spmd(nc, in_maps, list(range(N_CORES)))
    out = np.empty((B, S, D), np.float32)
    for core in range(N_CORES):
        bb = core // 2
        out[bb, qsels[core]] = res.results[core]["yT"].T
    return out

